# revision 2
# baseline (speedup 1.0000x reference)
"""BoundaryMaxPooling (segment range-max) Trainium2 kernel.

out[b, c, k] = max over t in [floor(seg[b,k,0]), floor(seg[b,k,1])] of x[b, c, t]

Strategy (8 NeuronCores, SPMD, no cross-core comm):
  - Shard: core g handles batch b = g//2, channel half ch = g%2 -> x shard
    [256, 256], its batch's segments, output shard [256, 504].
  - Device algorithm per 128-channel tile: build a ragged 8-level sparse
    table over T (level j = running max of 2^j-windows, only valid starts
    kept, so levels pack to 1801 columns with no tail fixups), compute per
    query k the level j = floor(log2(e-s)) (exact, via f32 exponent bits)
    and the two window start indices, then one GpSimd ap_gather fetches
    both windows for all queries; a final DVE max combines them.
  - Host work is layout-only: shard/pad inputs, put segment boundaries in
    the 16-partition-wrapped layout ap_gather uses for its index operand,
    and reassemble output shards. All arithmetic runs on device.
"""

import numpy as np

B, C, T, K = 4, 512, 256, 504
KP = 512  # queries padded to a multiple of 16 for the wrapped idx layout
# Ragged sparse table: level j holds max over [t, t+2^j-1] for t in [0, 257-2^j)
LVL_LEN = [257 - (1 << j) for j in range(8)]  # [256,255,253,249,241,225,193,129]
LVL_OFF = [sum(LVL_LEN[:j]) for j in range(8)]  # [0,256,511,764,1013,1254,1479,1672]
TBL = sum(LVL_LEN)  # 1801

_NC_CACHE = {}


def _build():
    from concourse import bacc, mybir
    import concourse.tile as tile

    op = mybir.AluOpType
    f32, i32, i16 = mybir.dt.float32, mybir.dt.int32, mybir.dt.int16

    nc = bacc.Bacc("TRN2", target_bir_lowering=False, debug=False, num_devices=8)
    x = nc.dram_tensor("x", [256, T], f32, kind="ExternalInput")
    segw = nc.dram_tensor("segw", [128, 64], f32, kind="ExternalInput")
    out = nc.dram_tensor("out", [256, KP], f32, kind="ExternalOutput")

    with tile.TileContext(nc) as tc:
        with tc.tile_pool(name="p", bufs=1) as pool:
            # --- query index computation (all [128, 32]-shaped, wrapped layout) ---
            sw = pool.tile([128, 64], f32, tag="sw")
            nc.sync.dma_start(out=sw[:, :], in_=segw[:, :])

            # floor(x) for x >= 0: rnd = RNE-round via +2^23-2^23, then -1 where rounded up
            rnd = pool.tile([128, 64], f32, tag="rnd")
            nc.vector.tensor_scalar(
                out=rnd[:, :], in0=sw[:, :], scalar1=float(2**23),
                scalar2=float(2**23), op0=op.add, op1=op.subtract)
            gt = pool.tile([128, 64], f32, tag="gt")
            nc.vector.tensor_tensor(out=gt[:, :], in0=rnd[:, :], in1=sw[:, :], op=op.is_gt)
            fl = pool.tile([128, 64], f32, tag="fl")
            nc.vector.tensor_tensor(out=fl[:, :], in0=rnd[:, :], in1=gt[:, :], op=op.subtract)
            s = fl[:, 0:32]
            e = pool.tile([128, 32], f32, tag="e")
            nc.vector.tensor_tensor(out=e[:, :], in0=fl[:, 32:64], in1=s, op=op.max)
            len1 = pool.tile([128, 32], f32, tag="len1")
            nc.vector.tensor_tensor(out=len1[:, :], in0=e[:, :], in1=s, op=op.subtract)
            # j = clamp(exponent(e-s), 0) -- exact for integer-valued f32
            # (bitwise and arith alu ops cannot fuse in one tensor_scalar)
            sh = pool.tile([128, 32], i32, tag="sh")
            nc.vector.tensor_scalar(
                out=sh[:, :], in0=len1[:, :].bitcast(i32), scalar1=23, scalar2=None,
                op0=op.logical_shift_right)
            mx = pool.tile([128, 32], i32, tag="mx")  # biased exponent, >= 127
            nc.vector.tensor_scalar(out=mx[:, :], in0=sh[:, :], scalar1=127, scalar2=None, op0=op.max)
            p2i = pool.tile([128, 32], i32, tag="p2i")  # bits of 2.0**j
            nc.vector.tensor_scalar(
                out=p2i[:, :], in0=mx[:, :], scalar1=23, scalar2=None,
                op0=op.logical_shift_left)
            jf = pool.tile([128, 32], f32, tag="jf")  # j + 127 as float
            nc.vector.tensor_copy(out=jf[:, :], in_=mx[:, :])
            pm1 = pool.tile([128, 32], f32, tag="pm1")  # 2^j - 1
            nc.vector.tensor_scalar(
                out=pm1[:, :], in0=p2i[:, :].bitcast(f32), scalar1=1.0, scalar2=None,
                op0=op.subtract)
            # level offset: LVL_OFF[j] = 257*j - (2^j - 1); a257 = 257*j
            a257 = pool.tile([128, 32], f32, tag="a257")
            nc.vector.tensor_scalar(
                out=a257[:, :], in0=jf[:, :], scalar1=257.0, scalar2=float(257 * 127),
                op0=op.mult, op1=op.subtract)
            b1 = pool.tile([128, 32], f32, tag="b1")
            nc.vector.tensor_tensor(out=b1[:, :], in0=a257[:, :], in1=s, op=op.add)
            idxf = pool.tile([128, 64], f32, tag="idxf")
            nc.vector.tensor_tensor(out=idxf[:, 0:32], in0=b1[:, :], in1=pm1[:, :], op=op.subtract)
            b2 = pool.tile([128, 32], f32, tag="b2")
            nc.vector.tensor_tensor(out=b2[:, :], in0=a257[:, :], in1=e[:, :], op=op.add)
            pm2 = pool.tile([128, 32], f32, tag="pm2")
            nc.vector.tensor_scalar(out=pm2[:, :], in0=pm1[:, :], scalar1=2.0, scalar2=None, op0=op.mult)
            nc.vector.tensor_tensor(out=idxf[:, 32:64], in0=b2[:, :], in1=pm2[:, :], op=op.subtract)
            idx = pool.tile([128, 64], i16, tag="idx")
            nc.vector.tensor_copy(out=idx[:, :], in_=idxf[:, :])

            # --- per 128-channel tile: table build, gather, combine ---
            for ct in range(2):
                tbl = pool.tile([128, TBL], f32, tag=f"tbl{ct}")
                nc.sync.dma_start(out=tbl[:, 0:T], in_=x[128 * ct:128 * (ct + 1), :])
                for j in range(1, 8):
                    d = 1 << (j - 1)
                    nc.vector.tensor_tensor(
                        out=tbl[:, LVL_OFF[j]:LVL_OFF[j] + LVL_LEN[j]],
                        in0=tbl[:, LVL_OFF[j - 1]:LVL_OFF[j - 1] + LVL_LEN[j]],
                        in1=tbl[:, LVL_OFF[j - 1] + d:LVL_OFF[j - 1] + d + LVL_LEN[j]],
                        op=op.max)
                g = pool.tile([128, 2 * KP], f32, tag=f"g{ct}")
                nc.gpsimd.ap_gather(g[:, :], tbl[:, :], idx[:, :],
                                    channels=128, num_elems=TBL, d=1, num_idxs=2 * KP)
                ot = pool.tile([128, KP], f32, tag=f"ot{ct}")
                nc.vector.tensor_tensor(out=ot[:, :], in0=g[:, 0:KP], in1=g[:, KP:2 * KP], op=op.max)
                nc.sync.dma_start(out=out[128 * ct:128 * (ct + 1), :], in_=ot[:, :])
    nc.compile()
    return nc


def _get_nc():
    if "nc" not in _NC_CACHE:
        _NC_CACHE["nc"] = _build()
    return _NC_CACHE["nc"]


def _make_in_maps(input, segments):
    input = np.ascontiguousarray(input, dtype=np.float32)
    segments = np.ascontiguousarray(segments, dtype=np.float32)
    in_maps = []
    for g in range(8):
        b, ch = g // 2, g % 2
        xs = np.ascontiguousarray(input[b, ch * 256:(ch + 1) * 256, :])
        seg = np.zeros((KP, 2), np.float32)
        seg[:K] = segments[b]
        # wrapped layout: tile[q, f] = seg[16f + q]; replicated to all 8
        # 16-partition groups; s in cols 0:32, e in cols 32:64
        ss = np.tile(seg[:, 0].reshape(KP // 16, 16).T, (8, 1))
        ee = np.tile(seg[:, 1].reshape(KP // 16, 16).T, (8, 1))
        sw = np.ascontiguousarray(np.concatenate([ss, ee], axis=1), np.float32)
        in_maps.append({"x": xs, "segw": sw})
    return in_maps


def _assemble(results):
    outf = np.empty((B, C, K), np.float32)
    for g in range(8):
        b, ch = g // 2, g % 2
        outf[b, ch * 256:(ch + 1) * 256, :] = results[g]["out"][:, :K]
    return outf


def kernel(input, segments):
    from concourse.bass_utils import run_bass_kernel_spmd

    nc = _get_nc()
    in_maps = _make_in_maps(input, segments)
    res = run_bass_kernel_spmd(nc, in_maps, list(range(8)))
    return _assemble(res.results)


# revision 4
# speedup vs baseline: 1.2332x; 1.2332x over previous
"""BoundaryMaxPooling (segment range-max) Trainium2 kernel.

out[b, c, k] = max over t in [floor(seg[b,k,0]), floor(seg[b,k,1])] of x[b, c, t]

Strategy (8 NeuronCores, SPMD, no cross-core comm):
  - Shard: core g handles batch b = g//2, channel half ch = g%2 -> x shard
    [256, 256], its batch's segments, output shard [256, 504].
  - Per 128-channel tile, build a ragged 8-level sparse table over T on DVE
    (level j = running max of 2^j-windows; only valid window starts kept, so
    the levels pack into 1801 columns with no tail fixups).
  - Per query k, compute level j = floor(log2(e-s)) (exact, via f32 exponent
    bits) and the two covering-window start indices, entirely on DVE.
  - The per-query selection is a row gather: transpose the table to [t, c]
    layout (PE transpose -> PSUM -> ScalarE evac -> DMA to DRAM), then one
    SWDGE dma_gather per window fetches row (level,start) for all queries at
    once; DVE max of the two gathered planes is the answer in [k, c] layout.
  - Host work is layout-only: shard/pad inputs, place segment boundaries in
    the 16-partition-wrapped layout the gather index operand uses, transpose
    the [k, c] result back to [c, k], reassemble shards.
"""

import numpy as np

B, C, T, K = 4, 512, 256, 504
KP = 512  # queries padded to a multiple of 128
# Ragged sparse table: level j holds max over [t, t+2^j-1] for t in [0, 257-2^j)
LVL_LEN = [257 - (1 << j) for j in range(8)]
LVL_OFF = [sum(LVL_LEN[:j]) for j in range(8)]
TBL = sum(LVL_LEN)  # 1801
CSH = 256  # channels per core

_NC_CACHE = {}


def _build():
    from concourse import bacc, mybir
    import concourse.tile as tile

    op = mybir.AluOpType
    f32, i32, i16 = mybir.dt.float32, mybir.dt.int32, mybir.dt.int16

    nc = bacc.Bacc("TRN2", target_bir_lowering=False, debug=False, num_devices=8)
    x = nc.dram_tensor("x", [CSH, T], f32, kind="ExternalInput")
    segw = nc.dram_tensor("segw", [128, 64], f32, kind="ExternalInput")
    out = nc.dram_tensor("out", [128, 4 * CSH], f32, kind="ExternalOutput")
    tbl_dram = nc.dram_tensor("tbl_dram", [TBL, CSH], f32)

    with tile.TileContext(nc) as tc:
        with (
            tc.tile_pool(name="p", bufs=1) as pool,
            tc.tile_pool(name="ps", bufs=8, space="PSUM") as psum,
        ):
            # --- constant: 128x128 f32 identity for PE transpose ---
            ident_i = pool.tile([128, 128], i32, tag="ident_i")
            nc.gpsimd.iota(ident_i[:, :], pattern=[[1, 128]], base=0, channel_multiplier=-1)
            ident = pool.tile([128, 128], f32, tag="ident")
            nc.vector.tensor_scalar(out=ident[:, :], in0=ident_i[:, :], scalar1=0,
                                    scalar2=None, op0=op.is_equal)

            # --- query index computation (wrapped [128, 32] layout) ---
            sw = pool.tile([128, 64], f32, tag="sw")
            nc.sync.dma_start(out=sw[:, :], in_=segw[:, :])

            # floor(x) for x >= 0: RNE-round via +2^23-2^23, then -1 where rounded up
            rnd = pool.tile([128, 64], f32, tag="rnd")
            nc.vector.tensor_scalar(
                out=rnd[:, :], in0=sw[:, :], scalar1=float(2**23),
                scalar2=float(2**23), op0=op.add, op1=op.subtract)
            gt = pool.tile([128, 64], f32, tag="gt")
            nc.vector.tensor_tensor(out=gt[:, :], in0=rnd[:, :], in1=sw[:, :], op=op.is_gt)
            fl = pool.tile([128, 64], f32, tag="fl")
            nc.vector.tensor_tensor(out=fl[:, :], in0=rnd[:, :], in1=gt[:, :], op=op.subtract)
            s = fl[:, 0:32]
            e = pool.tile([128, 32], f32, tag="e")
            nc.vector.tensor_tensor(out=e[:, :], in0=fl[:, 32:64], in1=s, op=op.max)
            len1 = pool.tile([128, 32], f32, tag="len1")
            nc.vector.tensor_tensor(out=len1[:, :], in0=e[:, :], in1=s, op=op.subtract)
            # j = clamp(exponent(e-s), 0): exact for integer-valued f32
            sh = pool.tile([128, 32], i32, tag="sh")
            nc.vector.tensor_scalar(
                out=sh[:, :], in0=len1[:, :].bitcast(i32), scalar1=23, scalar2=None,
                op0=op.logical_shift_right)
            mx = pool.tile([128, 32], i32, tag="mx")  # biased exponent >= 127
            nc.vector.tensor_scalar(out=mx[:, :], in0=sh[:, :], scalar1=127, scalar2=None, op0=op.max)
            p2i = pool.tile([128, 32], i32, tag="p2i")  # bits of 2.0**j
            nc.vector.tensor_scalar(
                out=p2i[:, :], in0=mx[:, :], scalar1=23, scalar2=None,
                op0=op.logical_shift_left)
            jf = pool.tile([128, 32], f32, tag="jf")  # j + 127 as float
            nc.vector.tensor_copy(out=jf[:, :], in_=mx[:, :])
            pm1 = pool.tile([128, 32], f32, tag="pm1")  # 2^j - 1
            nc.vector.tensor_scalar(
                out=pm1[:, :], in0=p2i[:, :].bitcast(f32), scalar1=1.0, scalar2=None,
                op0=op.subtract)
            # level offset: LVL_OFF[j] = 257*j - (2^j - 1); a257 = 257*j
            a257 = pool.tile([128, 32], f32, tag="a257")
            nc.vector.tensor_scalar(
                out=a257[:, :], in0=jf[:, :], scalar1=257.0, scalar2=float(257 * 127),
                op0=op.mult, op1=op.subtract)
            b1 = pool.tile([128, 32], f32, tag="b1")
            nc.vector.tensor_tensor(out=b1[:, :], in0=a257[:, :], in1=s, op=op.add)
            idx1f = pool.tile([128, 32], f32, tag="idx1f")
            nc.vector.tensor_tensor(out=idx1f[:, :], in0=b1[:, :], in1=pm1[:, :], op=op.subtract)
            b2 = pool.tile([128, 32], f32, tag="b2")
            nc.vector.tensor_tensor(out=b2[:, :], in0=a257[:, :], in1=e[:, :], op=op.add)
            pm2 = pool.tile([128, 32], f32, tag="pm2")
            nc.vector.tensor_scalar(out=pm2[:, :], in0=pm1[:, :], scalar1=2.0, scalar2=None, op0=op.mult)
            idx2f = pool.tile([128, 32], f32, tag="idx2f")
            nc.vector.tensor_tensor(out=idx2f[:, :], in0=b2[:, :], in1=pm2[:, :], op=op.subtract)
            idx1 = pool.tile([128, 32], i16, tag="idx1")
            nc.vector.tensor_copy(out=idx1[:, :], in_=idx1f[:, :])
            idx2 = pool.tile([128, 32], i16, tag="idx2")
            nc.vector.tensor_copy(out=idx2[:, :], in_=idx2f[:, :])

            # --- sparse table build per 128-channel tile ([c, t] layout) ---
            tbls = []
            for ct in range(2):
                tbl = pool.tile([128, TBL], f32, tag=f"tbl{ct}")
                nc.sync.dma_start(out=tbl[:, 0:T], in_=x[128 * ct:128 * (ct + 1), :])
                for j in range(1, 8):
                    d = 1 << (j - 1)
                    nc.vector.tensor_tensor(
                        out=tbl[:, LVL_OFF[j]:LVL_OFF[j] + LVL_LEN[j]],
                        in0=tbl[:, LVL_OFF[j - 1]:LVL_OFF[j - 1] + LVL_LEN[j]],
                        in1=tbl[:, LVL_OFF[j - 1] + d:LVL_OFF[j - 1] + d + LVL_LEN[j]],
                        op=op.max)
                tbls.append(tbl)

            # --- transpose table to [t, c] in DRAM: PE -> PSUM -> ACT -> DMA ---
            chunks = []  # (t0, m) column chunks of the [c, t]-layout table
            t0 = 0
            while t0 < TBL:
                m = min(128, TBL - t0)
                chunks.append((t0, m))
                t0 += m
            GRP = 4  # transpose chunks per PSUM bank / evac / DMA
            for ct in range(2):
                for g0 in range(0, len(chunks), GRP):
                    grp = chunks[g0:g0 + GRP]
                    pt = psum.tile([128, 512], f32, tag="pt")
                    ev = pool.tile([128, 512], f32, tag=f"ev{ct}_{g0}")
                    for r, (t0, m) in enumerate(grp):
                        nc.tensor.transpose(
                            pt[0:m, 128 * r:128 * (r + 1)],
                            tbls[ct][:, t0:t0 + m], ident[:, :])
                        nc.scalar.copy(out=ev[0:m, 128 * r:128 * (r + 1)],
                                       in_=pt[0:m, 128 * r:128 * (r + 1)])
                    # DMA rows t0+p (p<m) of each chunk to tbl_dram[t, 128*ct:128*ct+128]
                    for r, (t0, m) in enumerate(grp):
                        nc.sync.dma_start(
                            out=tbl_dram[t0:t0 + m, 128 * ct:128 * (ct + 1)],
                            in_=ev[0:m, 128 * r:128 * (r + 1)])

            # --- gather both windows for all queries: rows of tbl_dram ---
            gs = []
            for w, idxw in enumerate((idx1, idx2)):
                g = pool.tile([128, 4, CSH], f32, tag=f"g{w}")
                nc.gpsimd.dma_gather(
                    g[:, :, :], tbl_dram[:, :], idxw[:, :],
                    num_idxs=KP, num_idxs_reg=KP, elem_size=CSH, queue_num=0)
                gs.append(g)
            res = pool.tile([128, 4 * CSH], f32, tag="res")
            nc.vector.tensor_tensor(
                out=res[:, :].rearrange("p (q c) -> p q c", q=4),
                in0=gs[0][:, :, :], in1=gs[1][:, :, :], op=op.max)
            nc.sync.dma_start(out=out[:, :], in_=res[:, :])
    nc.compile()
    return nc


def _get_nc():
    if "nc" not in _NC_CACHE:
        _NC_CACHE["nc"] = _build()
    return _NC_CACHE["nc"]


def _make_in_maps(input, segments):
    input = np.ascontiguousarray(input, dtype=np.float32)
    segments = np.ascontiguousarray(segments, dtype=np.float32)
    in_maps = []
    for g in range(8):
        b, ch = g // 2, g % 2
        xs = np.ascontiguousarray(input[b, ch * CSH:(ch + 1) * CSH, :])
        seg = np.zeros((KP, 2), np.float32)
        seg[:K] = segments[b]
        # wrapped layout: tile[q, f] = seg[16f + q]; replicated to all 8
        # 16-partition groups; s in cols 0:32, e in cols 32:64
        ss = np.tile(seg[:, 0].reshape(KP // 16, 16).T, (8, 1))
        ee = np.tile(seg[:, 1].reshape(KP // 16, 16).T, (8, 1))
        sw = np.ascontiguousarray(np.concatenate([ss, ee], axis=1), np.float32)
        in_maps.append({"x": xs, "segw": sw})
    return in_maps


def _assemble(results):
    outf = np.empty((B, C, K), np.float32)
    for g in range(8):
        b, ch = g // 2, g % 2
        # res[p, q, c] = answer for query k = q*128 + p, channel c
        arr = results[g]["out"].reshape(128, 4, CSH)
        ans_kc = arr.transpose(1, 0, 2).reshape(KP, CSH)  # [k, c]
        outf[b, ch * CSH:(ch + 1) * CSH, :] = ans_kc[:K, :].T
    return outf


def kernel(input, segments):
    from concourse.bass_utils import run_bass_kernel_spmd

    nc = _get_nc()
    in_maps = _make_in_maps(input, segments)
    res = run_bass_kernel_spmd(nc, in_maps, list(range(8)))
    return _assemble(res.results)


# revision 6
# speedup vs baseline: 1.4456x; 1.1723x over previous
"""BoundaryMaxPooling (segment range-max) Trainium2 kernel.

out[b, c, k] = max over t in [floor(seg[b,k,0]), floor(seg[b,k,1])] of x[b, c, t]

Strategy (8 NeuronCores, SPMD, no cross-core comm):
  - Shard: core g handles batch b = g//2, channel half ch = g%2 -> x shard
    [256, 256], its batch's segments, output shard [256, 504].
  - Per 128-channel tile, build a ragged 8-level sparse table over T on DVE
    (level j = running max of 2^j-windows; only valid window starts kept, so
    the levels pack into 1801 columns with no tail fixups).
  - Per query k, compute level j = floor(log2(e-s)) (exact, via f32 exponent
    bits) and the two covering-window start indices, entirely on DVE.
  - The per-query selection is a row gather: transpose the table to [t, c]
    layout (PE transpose -> PSUM -> ScalarE evac -> DMA to DRAM), then one
    SWDGE dma_gather per window fetches row (level,start) for all queries at
    once; DVE max of the two gathered planes is the answer in [k, c] layout.
  - Host work is layout-only: shard/pad inputs, place segment boundaries in
    the 16-partition-wrapped layout the gather index operand uses, transpose
    the [k, c] result back to [c, k], reassemble shards.
"""

import numpy as np

B, C, T, K = 4, 512, 256, 504
KP = 512  # queries padded to a multiple of 128
# Ragged sparse table: level j holds max over [t, t+2^j-1] for t in [0, 257-2^j)
LVL_LEN = [257 - (1 << j) for j in range(8)]
LVL_OFF = [sum(LVL_LEN[:j]) for j in range(8)]
TBL = sum(LVL_LEN)  # 1801
CSH = 256  # channels per core

_NC_CACHE = {}


def _build():
    from concourse import bacc, mybir
    import concourse.tile as tile

    op = mybir.AluOpType
    f32, i32, i16 = mybir.dt.float32, mybir.dt.int32, mybir.dt.int16

    nc = bacc.Bacc("TRN2", target_bir_lowering=False, debug=False, num_devices=8)
    x = nc.dram_tensor("x", [CSH, T], f32, kind="ExternalInput")
    segw = nc.dram_tensor("segw", [128, 64], f32, kind="ExternalInput")
    out = nc.dram_tensor("out", [128, 4 * CSH], f32, kind="ExternalOutput")
    tbl_dram = nc.dram_tensor("tbl_dram", [TBL, CSH], f32)

    with tile.TileContext(nc) as tc:
        with (
            tc.tile_pool(name="p", bufs=1) as pool,
            tc.tile_pool(name="ps", bufs=8, space="PSUM") as psum,
        ):
            # --- constant: 128x128 f32 identity for PE transpose ---
            ident_i = pool.tile([128, 128], i32, tag="ident_i")
            nc.gpsimd.iota(ident_i[:, :], pattern=[[1, 128]], base=0, channel_multiplier=-1)
            ident = pool.tile([128, 128], f32, tag="ident")
            nc.vector.tensor_scalar(out=ident[:, :], in0=ident_i[:, :], scalar1=0,
                                    scalar2=None, op0=op.is_equal)

            # --- query index computation (wrapped [128, 32] layout) ---
            sw = pool.tile([128, 64], f32, tag="sw")
            nc.sync.dma_start(out=sw[:, :], in_=segw[:, :])

            # floor(x) for x >= 0: RNE-round via +2^23-2^23, then -1 where rounded up
            rnd = pool.tile([128, 64], f32, tag="rnd")
            nc.vector.tensor_scalar(
                out=rnd[:, :], in0=sw[:, :], scalar1=float(2**23),
                scalar2=float(2**23), op0=op.add, op1=op.subtract)
            gt = pool.tile([128, 64], f32, tag="gt")
            nc.vector.tensor_tensor(out=gt[:, :], in0=rnd[:, :], in1=sw[:, :], op=op.is_gt)
            fl = pool.tile([128, 64], f32, tag="fl")
            nc.vector.tensor_tensor(out=fl[:, :], in0=rnd[:, :], in1=gt[:, :], op=op.subtract)
            s = fl[:, 0:32]
            e = pool.tile([128, 32], f32, tag="e")
            nc.vector.tensor_tensor(out=e[:, :], in0=fl[:, 32:64], in1=s, op=op.max)
            len1 = pool.tile([128, 32], f32, tag="len1")
            nc.vector.tensor_tensor(out=len1[:, :], in0=e[:, :], in1=s, op=op.subtract)
            # j = clamp(exponent(e-s), 0): exact for integer-valued f32
            sh = pool.tile([128, 32], i32, tag="sh")
            nc.vector.tensor_scalar(
                out=sh[:, :], in0=len1[:, :].bitcast(i32), scalar1=23, scalar2=None,
                op0=op.logical_shift_right)
            mx = pool.tile([128, 32], i32, tag="mx")  # biased exponent >= 127
            nc.vector.tensor_scalar(out=mx[:, :], in0=sh[:, :], scalar1=127, scalar2=None, op0=op.max)
            p2i = pool.tile([128, 32], i32, tag="p2i")  # bits of 2.0**j
            nc.vector.tensor_scalar(
                out=p2i[:, :], in0=mx[:, :], scalar1=23, scalar2=None,
                op0=op.logical_shift_left)
            jf = pool.tile([128, 32], f32, tag="jf")  # j + 127 as float
            nc.vector.tensor_copy(out=jf[:, :], in_=mx[:, :])
            pm1 = pool.tile([128, 32], f32, tag="pm1")  # 2^j - 1
            nc.vector.tensor_scalar(
                out=pm1[:, :], in0=p2i[:, :].bitcast(f32), scalar1=1.0, scalar2=None,
                op0=op.subtract)
            # level offset: LVL_OFF[j] = 257*j - (2^j - 1); a257 = 257*j
            a257 = pool.tile([128, 32], f32, tag="a257")
            nc.vector.tensor_scalar(
                out=a257[:, :], in0=jf[:, :], scalar1=257.0, scalar2=float(257 * 127),
                op0=op.mult, op1=op.subtract)
            b1 = pool.tile([128, 32], f32, tag="b1")
            nc.vector.tensor_tensor(out=b1[:, :], in0=a257[:, :], in1=s, op=op.add)
            idx1f = pool.tile([128, 32], f32, tag="idx1f")
            nc.vector.tensor_tensor(out=idx1f[:, :], in0=b1[:, :], in1=pm1[:, :], op=op.subtract)
            b2 = pool.tile([128, 32], f32, tag="b2")
            nc.vector.tensor_tensor(out=b2[:, :], in0=a257[:, :], in1=e[:, :], op=op.add)
            pm2 = pool.tile([128, 32], f32, tag="pm2")
            nc.vector.tensor_scalar(out=pm2[:, :], in0=pm1[:, :], scalar1=2.0, scalar2=None, op0=op.mult)
            idx2f = pool.tile([128, 32], f32, tag="idx2f")
            nc.vector.tensor_tensor(out=idx2f[:, :], in0=b2[:, :], in1=pm2[:, :], op=op.subtract)
            idx1 = pool.tile([128, 32], i16, tag="idx1")
            nc.vector.tensor_copy(out=idx1[:, :], in_=idx1f[:, :])
            idx2 = pool.tile([128, 32], i16, tag="idx2")
            nc.vector.tensor_copy(out=idx2[:, :], in_=idx2f[:, :])

            # --- sparse table build per 128-channel tile ([c, t] layout) ---
            tbls = []
            for ct in range(2):
                tbl = pool.tile([128, TBL], f32, tag=f"tbl{ct}")
                nc.sync.dma_start(out=tbl[:, 0:T], in_=x[128 * ct:128 * (ct + 1), :])
                for j in range(1, 8):
                    d = 1 << (j - 1)
                    nc.vector.tensor_tensor(
                        out=tbl[:, LVL_OFF[j]:LVL_OFF[j] + LVL_LEN[j]],
                        in0=tbl[:, LVL_OFF[j - 1]:LVL_OFF[j - 1] + LVL_LEN[j]],
                        in1=tbl[:, LVL_OFF[j - 1] + d:LVL_OFF[j - 1] + d + LVL_LEN[j]],
                        op=op.max)
                tbls.append(tbl)

            # --- transpose table to [t, c] in DRAM: PE -> PSUM -> ACT -> DMA ---
            chunks = []  # (t0, m) column chunks of the [c, t]-layout table
            t0 = 0
            while t0 < TBL:
                m = min(128, TBL - t0)
                chunks.append((t0, m))
                t0 += m
            GRP = 4  # transpose chunks per PSUM bank / evac / DMA
            for ct in range(2):
                for g0 in range(0, len(chunks), GRP):
                    grp = chunks[g0:g0 + GRP]
                    pt = psum.tile([128, 512], f32, tag="pt")
                    ev = pool.tile([128, 512], f32, tag=f"ev{ct}_{g0}")
                    for r, (t0, m) in enumerate(grp):
                        nc.tensor.transpose(
                            pt[0:m, 128 * r:128 * (r + 1)],
                            tbls[ct][:, t0:t0 + m], ident[:, :])
                    # one evac + one DMA per group (fewer, bigger ops)
                    nfull = sum(1 for (_, m) in grp if m == 128)
                    if nfull:
                        nc.scalar.copy(out=ev[:, 0:128 * nfull], in_=pt[:, 0:128 * nfull])
                        t0g = grp[0][0]
                        dst = tbl_dram[t0g:t0g + 128 * nfull, 128 * ct:128 * (ct + 1)]
                        nc.sync.dma_start(
                            out=dst.rearrange("(a p) c -> p a c", p=128),
                            in_=ev[:, 0:128 * nfull].rearrange("p (a c) -> p a c", a=nfull))
                    for r, (t0, m) in enumerate(grp):
                        if m != 128:  # ragged tail chunk
                            nc.scalar.copy(out=ev[0:m, 128 * r:128 * (r + 1)],
                                           in_=pt[0:m, 128 * r:128 * (r + 1)])
                            nc.sync.dma_start(
                                out=tbl_dram[t0:t0 + m, 128 * ct:128 * (ct + 1)],
                                in_=ev[0:m, 128 * r:128 * (r + 1)])

            # --- gather both windows for all queries: rows of tbl_dram ---
            gs = []
            for w, idxw in enumerate((idx1, idx2)):
                g = pool.tile([128, 4, CSH], f32, tag=f"g{w}")
                nc.gpsimd.dma_gather(
                    g[:, :, :], tbl_dram[:, :], idxw[:, :],
                    num_idxs=KP, num_idxs_reg=KP, elem_size=CSH, queue_num=0)
                gs.append(g)
            res = pool.tile([128, 4 * CSH], f32, tag="res")
            nc.vector.tensor_tensor(
                out=res[:, :].rearrange("p (q c) -> p q c", q=4),
                in0=gs[0][:, :, :], in1=gs[1][:, :, :], op=op.max)
            nc.sync.dma_start(out=out[:, :], in_=res[:, :])
    nc.compile()
    return nc


def _get_nc():
    if "nc" not in _NC_CACHE:
        _NC_CACHE["nc"] = _build()
    return _NC_CACHE["nc"]


def _make_in_maps(input, segments):
    input = np.ascontiguousarray(input, dtype=np.float32)
    segments = np.ascontiguousarray(segments, dtype=np.float32)
    in_maps = []
    for g in range(8):
        b, ch = g // 2, g % 2
        xs = np.ascontiguousarray(input[b, ch * CSH:(ch + 1) * CSH, :])
        seg = np.zeros((KP, 2), np.float32)
        seg[:K] = segments[b]
        # wrapped layout: tile[q, f] = seg[16f + q]; replicated to all 8
        # 16-partition groups; s in cols 0:32, e in cols 32:64
        ss = np.tile(seg[:, 0].reshape(KP // 16, 16).T, (8, 1))
        ee = np.tile(seg[:, 1].reshape(KP // 16, 16).T, (8, 1))
        sw = np.ascontiguousarray(np.concatenate([ss, ee], axis=1), np.float32)
        in_maps.append({"x": xs, "segw": sw})
    return in_maps


def _assemble(results):
    outf = np.empty((B, C, K), np.float32)
    for g in range(8):
        b, ch = g // 2, g % 2
        # res[p, q, c] = answer for query k = q*128 + p, channel c
        arr = results[g]["out"].reshape(128, 4, CSH)
        ans_kc = arr.transpose(1, 0, 2).reshape(KP, CSH)  # [k, c]
        outf[b, ch * CSH:(ch + 1) * CSH, :] = ans_kc[:K, :].T
    return outf


def kernel(input, segments):
    from concourse.bass_utils import run_bass_kernel_spmd

    nc = _get_nc()
    in_maps = _make_in_maps(input, segments)
    res = run_bass_kernel_spmd(nc, in_maps, list(range(8)))
    return _assemble(res.results)


# revision 23
# speedup vs baseline: 2.1457x; 1.4843x over previous
"""BoundaryMaxPooling (segment range-max) Trainium2 kernel.

out[b, c, k] = max over t in [floor(seg[b,k,0]), floor(seg[b,k,1])] of x[b, c, t]

Strategy (8 NeuronCores, SPMD, no cross-core comm):
  - Shard: core g handles batch b = g//2, channel half ch = g%2 -> x shard
    [256, 256], its batch's segments, output shard [256, 504].
  - Per 128-channel tile, build a ragged 8-level sparse table over T on DVE
    (level j = running max of 2^j-windows; only valid window starts kept, so
    the levels pack into 1801 columns with no tail fixups).
  - Per query k, compute level j = floor(log2(e-s)) (exact, via f32 exponent
    bits) and the two covering-window start indices, entirely on DVE.
  - The per-query selection is a row gather: transpose the table to [t, c]
    layout (PE transpose -> PSUM -> ScalarE evac -> DMA to DRAM), then one
    SWDGE dma_gather per window fetches row (level,start) for all queries at
    once; DVE max of the two gathered planes is the answer in [k, c] layout.
  - Host work is layout-only: shard/pad inputs, place segment boundaries in
    the 16-partition-wrapped layout the gather index operand uses, transpose
    the [k, c] result back to [c, k], reassemble shards.
"""

import numpy as np

B, C, T, K = 4, 512, 256, 504
KP = 512  # queries padded to a multiple of 128
# Ragged sparse table: level j holds max over [t, t+2^j-1] for t in [0, 257-2^j)
LVL_LEN = [257 - (1 << j) for j in range(8)]
LVL_OFF = [sum(LVL_LEN[:j]) for j in range(8)]
TBL = sum(LVL_LEN)  # 1801
CSH = 256  # channels per core

_NC_CACHE = {}


def _build():
    from concourse import bacc, mybir
    import concourse.tile as tile

    op = mybir.AluOpType
    f32, i32, i16 = mybir.dt.float32, mybir.dt.int32, mybir.dt.int16

    nc = bacc.Bacc("TRN2", target_bir_lowering=False, debug=False, num_devices=8)
    x = nc.dram_tensor("x", [CSH, T], f32, kind="ExternalInput")
    segw = nc.dram_tensor("segw", [128, 64], f32, kind="ExternalInput")
    out = nc.dram_tensor("out", [128, 4 * CSH], f32, kind="ExternalOutput")
    tbl_dram = nc.dram_tensor("tbl_dram", [TBL, CSH], f32)
    # Alias of tbl_dram (same DRAM address, distinct name): the gather preps
    # read through the alias so Tile doesn't see a read of tbl_dram that the
    # later table-write DMAs would have to WAR-wait on (the prep's "read"
    # happens only at trigger time; ordering is enforced by tsem below).
    tbl_alias = nc.dram_tensor("tbl_alias", [TBL, CSH], f32)
    nc.lookup_mloc(tbl_alias).addr = nc.lookup_mloc(tbl_dram).addr

    with tile.TileContext(nc) as tc:
        with (
            tc.tile_pool(name="p", bufs=1) as pool,
            tc.tile_pool(name="ps", bufs=8, space="PSUM") as psum,
        ):
            # --- constant: 128x128 f32 identity for PE transpose ---
            ident_i = pool.tile([128, 128], i32, tag="ident_i")
            nc.gpsimd.iota(ident_i[:, :], pattern=[[1, 128]], base=0, channel_multiplier=-1)
            ident = pool.tile([128, 128], f32, tag="ident")
            nc.vector.tensor_scalar(out=ident[:, :], in0=ident_i[:, :], scalar1=0,
                                    scalar2=None, op0=op.is_equal)

            # --- query index computation (wrapped [128, 32] layout) ---
            sw = pool.tile([128, 64], f32, tag="sw")
            nc.sync.dma_start(out=sw[:, :], in_=segw[:, :])

            # floor(x) for x >= 0: RNE-round via +2^23-2^23, then -1 where rounded up
            rnd = pool.tile([128, 64], f32, tag="rnd")
            nc.vector.tensor_scalar(
                out=rnd[:, :], in0=sw[:, :], scalar1=float(2**23),
                scalar2=float(2**23), op0=op.add, op1=op.subtract)
            gt = pool.tile([128, 64], f32, tag="gt")
            nc.vector.tensor_tensor(out=gt[:, :], in0=rnd[:, :], in1=sw[:, :], op=op.is_gt)
            fl = pool.tile([128, 64], f32, tag="fl")
            nc.vector.tensor_tensor(out=fl[:, :], in0=rnd[:, :], in1=gt[:, :], op=op.subtract)
            s = fl[:, 0:32]
            e = pool.tile([128, 32], f32, tag="e")
            nc.vector.tensor_tensor(out=e[:, :], in0=fl[:, 32:64], in1=s, op=op.max)
            len1 = pool.tile([128, 32], f32, tag="len1")
            nc.vector.tensor_tensor(out=len1[:, :], in0=e[:, :], in1=s, op=op.subtract)
            # j = clamp(exponent(e-s), 0): exact for integer-valued f32
            sh = pool.tile([128, 32], i32, tag="sh")
            nc.vector.tensor_scalar(
                out=sh[:, :], in0=len1[:, :].bitcast(i32), scalar1=23, scalar2=None,
                op0=op.logical_shift_right)
            mx = pool.tile([128, 32], i32, tag="mx")  # biased exponent >= 127
            nc.vector.tensor_scalar(out=mx[:, :], in0=sh[:, :], scalar1=127, scalar2=None, op0=op.max)
            p2i = pool.tile([128, 32], i32, tag="p2i")  # bits of 2.0**j
            nc.vector.tensor_scalar(
                out=p2i[:, :], in0=mx[:, :], scalar1=23, scalar2=None,
                op0=op.logical_shift_left)
            jf = pool.tile([128, 32], f32, tag="jf")  # j + 127 as float
            nc.vector.tensor_copy(out=jf[:, :], in_=mx[:, :])
            pm1 = pool.tile([128, 32], f32, tag="pm1")  # 2^j - 1
            nc.vector.tensor_scalar(
                out=pm1[:, :], in0=p2i[:, :].bitcast(f32), scalar1=1.0, scalar2=None,
                op0=op.subtract)
            # level offset: LVL_OFF[j] = 257*j - (2^j - 1); a257 = 257*j
            a257 = pool.tile([128, 32], f32, tag="a257")
            nc.vector.tensor_scalar(
                out=a257[:, :], in0=jf[:, :], scalar1=257.0, scalar2=float(257 * 127),
                op0=op.mult, op1=op.subtract)
            b1 = pool.tile([128, 32], f32, tag="b1")
            nc.vector.tensor_tensor(out=b1[:, :], in0=a257[:, :], in1=s, op=op.add)
            idx1f = pool.tile([128, 32], f32, tag="idx1f")
            nc.vector.tensor_tensor(out=idx1f[:, :], in0=b1[:, :], in1=pm1[:, :], op=op.subtract)
            b2 = pool.tile([128, 32], f32, tag="b2")
            nc.vector.tensor_tensor(out=b2[:, :], in0=a257[:, :], in1=e[:, :], op=op.add)
            pm2 = pool.tile([128, 32], f32, tag="pm2")
            nc.vector.tensor_scalar(out=pm2[:, :], in0=pm1[:, :], scalar1=2.0, scalar2=None, op0=op.mult)
            idx2f = pool.tile([128, 32], f32, tag="idx2f")
            nc.vector.tensor_tensor(out=idx2f[:, :], in0=b2[:, :], in1=pm2[:, :], op=op.subtract)
            idx1 = pool.tile([128, 32], i16, tag="idx1")
            nc.vector.tensor_copy(out=idx1[:, :], in_=idx1f[:, :])
            idx2 = pool.tile([128, 32], i16, tag="idx2")
            nc.vector.tensor_copy(out=idx2[:, :], in_=idx2f[:, :])

            # --- gather preps: SWDGE descriptor-gen runs now (only needs the
            # idx tiles); the table-read dep is deferred to trigger_dma ---
            dma_sem = nc.alloc_semaphore("gsem")
            gs = []
            for w, idxw in enumerate((idx1, idx2)):
                g = pool.tile([128, 4, CSH], f32, tag=f"g{w}")
                nc.gpsimd.dma_gather(
                    g[:, :, :], tbl_alias[:, :], idxw[:, :],
                    num_idxs=KP, num_idxs_reg=KP, elem_size=CSH,
                    prepare_only=True, sem=dma_sem, queue_num=0)
                gs.append(g)

            # --- sparse table build per 128-channel tile ([c, t] layout) ---
            tbls = []
            for ct in range(2):
                tbl = pool.tile([128, TBL], f32, tag=f"tbl{ct}")
                nc.sync.dma_start(out=tbl[:, 0:T], in_=x[128 * ct:128 * (ct + 1), :])
                for j in range(1, 8):
                    d = 1 << (j - 1)
                    nc.vector.tensor_tensor(
                        out=tbl[:, LVL_OFF[j]:LVL_OFF[j] + LVL_LEN[j]],
                        in0=tbl[:, LVL_OFF[j - 1]:LVL_OFF[j - 1] + LVL_LEN[j]],
                        in1=tbl[:, LVL_OFF[j - 1] + d:LVL_OFF[j - 1] + d + LVL_LEN[j]],
                        op=op.max)
                tbls.append(tbl)

            # --- transpose table to [t, c] in DRAM: PE -> PSUM -> ACT -> DMA ---
            chunks = []  # (t0, m) column chunks of the [c, t]-layout table
            t0 = 0
            while t0 < TBL:
                m = min(128, TBL - t0)
                chunks.append((t0, m))
                t0 += m
            GRP = 4  # transpose chunks per PSUM bank / evac / DMA
            tbl_writes = []
            for ct in range(2):
                for g0 in range(0, len(chunks), GRP):
                    grp = chunks[g0:g0 + GRP]
                    pt = psum.tile([128, 512], f32, tag="pt")
                    ev = pool.tile([128, 512], f32, tag=f"ev{ct}_{g0}")
                    for r, (t0, m) in enumerate(grp):
                        nc.tensor.transpose(
                            pt[0:m, 128 * r:128 * (r + 1)],
                            tbls[ct][:, t0:t0 + m], ident[:, :])
                    # one evac + one DMA per group (fewer, bigger ops)
                    nfull = sum(1 for (_, m) in grp if m == 128)
                    if nfull:
                        nc.scalar.copy(out=ev[:, 0:128 * nfull], in_=pt[:, 0:128 * nfull])
                        t0g = grp[0][0]
                        dst = tbl_dram[t0g:t0g + 128 * nfull, 128 * ct:128 * (ct + 1)]
                        tbl_writes.append(nc.sync.dma_start(
                            out=dst.rearrange("(a p) c -> p a c", p=128),
                            in_=ev[:, 0:128 * nfull].rearrange("p (a c) -> p a c", a=nfull)))
                    for r, (t0, m) in enumerate(grp):
                        if m != 128:  # ragged tail chunk
                            nc.scalar.copy(out=ev[0:m, 128 * r:128 * (r + 1)],
                                           in_=pt[0:m, 128 * r:128 * (r + 1)])
                            tbl_writes.append(nc.sync.dma_start(
                                out=tbl_dram[t0:t0 + m, 128 * ct:128 * (ct + 1)],
                                in_=ev[0:m, 128 * r:128 * (r + 1)]))

            # --- fire the prepared gathers once the table is in DRAM.
            # The preps were emitted before the table writes, so Tile's
            # deferred-RAW machinery never saw the producers; wire the
            # trigger's dependency on every table write explicitly. ---
            # Trigger ordering: a tiny DMA read of tbl_dram picks up RAW waits
            # on every table-write's DMA completion; the trigger then declares
            # a WAW "write" of the same tile via signals_writable, so Tile
            # fences it on that DMA's completion (= table fully in DRAM).
            conf = pool.tile([1, 64], f32, tag="conf")
            nc.sync.dma_start(out=conf[:, :], in_=tbl_dram[0:1, 0:64])
            nc.gpsimd.trigger_dma(count=None, signals_writable=[conf[:, :]])
            res = pool.tile([128, 4 * CSH], f32, tag="res")
            nc.vector.tensor_tensor(
                out=res[:, :].rearrange("p (q c) -> p q c", q=4),
                in0=gs[0][:, :, :], in1=gs[1][:, :, :], op=op.max)
            nc.sync.dma_start(out=out[:, :], in_=res[:, :])
    nc.compile()
    return nc


def _get_nc():
    if "nc" not in _NC_CACHE:
        _NC_CACHE["nc"] = _build()
    return _NC_CACHE["nc"]


def _make_in_maps(input, segments):
    input = np.ascontiguousarray(input, dtype=np.float32)
    segments = np.ascontiguousarray(segments, dtype=np.float32)
    in_maps = []
    for g in range(8):
        b, ch = g // 2, g % 2
        xs = np.ascontiguousarray(input[b, ch * CSH:(ch + 1) * CSH, :])
        seg = np.zeros((KP, 2), np.float32)
        seg[:K] = segments[b]
        # wrapped layout: tile[q, f] = seg[16f + q]; replicated to all 8
        # 16-partition groups; s in cols 0:32, e in cols 32:64
        ss = np.tile(seg[:, 0].reshape(KP // 16, 16).T, (8, 1))
        ee = np.tile(seg[:, 1].reshape(KP // 16, 16).T, (8, 1))
        sw = np.ascontiguousarray(np.concatenate([ss, ee], axis=1), np.float32)
        in_maps.append({"x": xs, "segw": sw})
    return in_maps


def _assemble(results):
    outf = np.empty((B, C, K), np.float32)
    for g in range(8):
        b, ch = g // 2, g % 2
        # res[p, q, c] = answer for query k = q*128 + p, channel c
        arr = results[g]["out"].reshape(128, 4, CSH)
        ans_kc = arr.transpose(1, 0, 2).reshape(KP, CSH)  # [k, c]
        outf[b, ch * CSH:(ch + 1) * CSH, :] = ans_kc[:K, :].T
    return outf


def kernel(input, segments):
    from concourse.bass_utils import run_bass_kernel_spmd

    nc = _get_nc()
    in_maps = _make_in_maps(input, segments)
    res = run_bass_kernel_spmd(nc, in_maps, list(range(8)))
    return _assemble(res.results)


# revision 26
# speedup vs baseline: 2.2526x; 1.0498x over previous
"""BoundaryMaxPooling (segment range-max) Trainium2 kernel.

out[b, c, k] = max over t in [floor(seg[b,k,0]), floor(seg[b,k,1])] of x[b, c, t]

Strategy (8 NeuronCores, SPMD, no cross-core comm):
  - Shard: core g handles batch b = g//2, channel half ch = g%2 -> x shard
    [256, 256], its batch's segments, output shard [256, 504].
  - Per 128-channel tile, build a ragged 8-level sparse table over T on DVE
    (level j = running max of 2^j-windows; only valid window starts kept, so
    the levels pack into 1801 columns with no tail fixups).
  - Per query k, compute level j = floor(log2(e-s)) (exact, via f32 exponent
    bits) and the two covering-window start indices, entirely on DVE.
  - The per-query selection is a row gather: transpose the table to [t, c]
    layout (PE transpose -> PSUM -> ScalarE evac -> DMA to DRAM), then one
    SWDGE dma_gather per window fetches row (level,start) for all queries at
    once; DVE max of the two gathered planes is the answer in [k, c] layout.
  - Host work is layout-only: shard/pad inputs, place segment boundaries in
    the 16-partition-wrapped layout the gather index operand uses, transpose
    the [k, c] result back to [c, k], reassemble shards.
"""

import numpy as np

B, C, T, K = 4, 512, 256, 504
KP = 512  # queries padded to a multiple of 128
# Ragged sparse table: level j holds max over [t, t+2^j-1] for t in [0, 257-2^j)
LVL_LEN = [257 - (1 << j) for j in range(8)]
LVL_OFF = [sum(LVL_LEN[:j]) for j in range(8)]
TBL = sum(LVL_LEN)  # 1801
CSH = 256  # channels per core

_NC_CACHE = {}


def _build():
    from concourse import bacc, mybir
    import concourse.tile as tile

    op = mybir.AluOpType
    f32, i32, i16 = mybir.dt.float32, mybir.dt.int32, mybir.dt.int16

    nc = bacc.Bacc("TRN2", target_bir_lowering=False, debug=False, num_devices=8)
    x = nc.dram_tensor("x", [CSH, T], f32, kind="ExternalInput")
    segw = nc.dram_tensor("segw", [128, 64], f32, kind="ExternalInput")
    out = nc.dram_tensor("out", [128, 4 * CSH], f32, kind="ExternalOutput")
    tbl_dram = nc.dram_tensor("tbl_dram", [TBL, CSH], f32)
    # Alias of tbl_dram (same DRAM address, distinct name): the gather preps
    # read through the alias so Tile doesn't see a read of tbl_dram that the
    # later table-write DMAs would have to WAR-wait on (the prep's "read"
    # happens only at trigger time; ordering is enforced by tsem below).
    tbl_alias = nc.dram_tensor("tbl_alias", [TBL, CSH], f32)
    nc.lookup_mloc(tbl_alias).addr = nc.lookup_mloc(tbl_dram).addr

    with tile.TileContext(nc) as tc:
        with (
            tc.tile_pool(name="p", bufs=1) as pool,
            tc.tile_pool(name="ps", bufs=8, space="PSUM") as psum,
        ):
            # --- constant: 128x128 f32 identity for PE transpose ---
            ident_i = pool.tile([128, 128], i32, tag="ident_i")
            nc.gpsimd.iota(ident_i[:, :], pattern=[[1, 128]], base=0, channel_multiplier=-1)
            ident = pool.tile([128, 128], f32, tag="ident")
            nc.vector.tensor_scalar(out=ident[:, :], in0=ident_i[:, :], scalar1=0,
                                    scalar2=None, op0=op.is_equal)

            # --- query index computation (wrapped [128, 32] layout) ---
            sw = pool.tile([128, 64], f32, tag="sw")
            nc.sync.dma_start(out=sw[:, :], in_=segw[:, :])

            # floor(x) for x >= 0: RNE-round via +2^23-2^23, then -1 where rounded up
            rnd = pool.tile([128, 64], f32, tag="rnd")
            nc.vector.tensor_scalar(
                out=rnd[:, :], in0=sw[:, :], scalar1=float(2**23),
                scalar2=float(2**23), op0=op.add, op1=op.subtract)
            gt = pool.tile([128, 64], f32, tag="gt")
            nc.vector.tensor_tensor(out=gt[:, :], in0=rnd[:, :], in1=sw[:, :], op=op.is_gt)
            fl = pool.tile([128, 64], f32, tag="fl")
            nc.vector.tensor_tensor(out=fl[:, :], in0=rnd[:, :], in1=gt[:, :], op=op.subtract)
            s = fl[:, 0:32]
            e = pool.tile([128, 32], f32, tag="e")
            nc.vector.tensor_tensor(out=e[:, :], in0=fl[:, 32:64], in1=s, op=op.max)
            len1 = pool.tile([128, 32], f32, tag="len1")
            nc.vector.tensor_tensor(out=len1[:, :], in0=e[:, :], in1=s, op=op.subtract)
            # j = clamp(exponent(e-s), 0): exact for integer-valued f32
            sh = pool.tile([128, 32], i32, tag="sh")
            nc.vector.tensor_scalar(
                out=sh[:, :], in0=len1[:, :].bitcast(i32), scalar1=23, scalar2=None,
                op0=op.logical_shift_right)
            mx = pool.tile([128, 32], i32, tag="mx")  # biased exponent >= 127
            nc.vector.tensor_scalar(out=mx[:, :], in0=sh[:, :], scalar1=127, scalar2=None, op0=op.max)
            p2i = pool.tile([128, 32], i32, tag="p2i")  # bits of 2.0**j
            nc.vector.tensor_scalar(
                out=p2i[:, :], in0=mx[:, :], scalar1=23, scalar2=None,
                op0=op.logical_shift_left)
            jf = pool.tile([128, 32], f32, tag="jf")  # j + 127 as float
            nc.vector.tensor_copy(out=jf[:, :], in_=mx[:, :])
            pm1 = pool.tile([128, 32], f32, tag="pm1")  # 2^j - 1
            nc.vector.tensor_scalar(
                out=pm1[:, :], in0=p2i[:, :].bitcast(f32), scalar1=1.0, scalar2=None,
                op0=op.subtract)
            # level offset: LVL_OFF[j] = 257*j - (2^j - 1); a257 = 257*j
            a257 = pool.tile([128, 32], f32, tag="a257")
            nc.vector.tensor_scalar(
                out=a257[:, :], in0=jf[:, :], scalar1=257.0, scalar2=float(257 * 127),
                op0=op.mult, op1=op.subtract)
            b1 = pool.tile([128, 32], f32, tag="b1")
            nc.vector.tensor_tensor(out=b1[:, :], in0=a257[:, :], in1=s, op=op.add)
            idx1f = pool.tile([128, 32], f32, tag="idx1f")
            nc.vector.tensor_tensor(out=idx1f[:, :], in0=b1[:, :], in1=pm1[:, :], op=op.subtract)
            b2 = pool.tile([128, 32], f32, tag="b2")
            nc.vector.tensor_tensor(out=b2[:, :], in0=a257[:, :], in1=e[:, :], op=op.add)
            pm2 = pool.tile([128, 32], f32, tag="pm2")
            nc.vector.tensor_scalar(out=pm2[:, :], in0=pm1[:, :], scalar1=2.0, scalar2=None, op0=op.mult)
            idx2f = pool.tile([128, 32], f32, tag="idx2f")
            nc.vector.tensor_tensor(out=idx2f[:, :], in0=b2[:, :], in1=pm2[:, :], op=op.subtract)
            idx1 = pool.tile([128, 32], i16, tag="idx1")
            nc.vector.tensor_copy(out=idx1[:, :], in_=idx1f[:, :])
            idx2 = pool.tile([128, 32], i16, tag="idx2")
            nc.vector.tensor_copy(out=idx2[:, :], in_=idx2f[:, :])

            # --- gather preps: SWDGE descriptor-gen runs now (only needs the
            # idx tiles); the table-read dep is deferred to trigger_dma ---
            dma_sem = nc.alloc_semaphore("gsem")
            gs = []
            for w, idxw in enumerate((idx1, idx2)):
                g = pool.tile([128, 4, CSH], f32, tag=f"g{w}")
                nc.gpsimd.dma_gather(
                    g[:, :, :], tbl_alias[:, :], idxw[:, :],
                    num_idxs=KP, num_idxs_reg=KP, elem_size=CSH,
                    prepare_only=True, sem=dma_sem, queue_num=0)
                gs.append(g)

            # --- sparse table build per 128-channel tile ([c, t] layout) ---
            tbls = []
            for ct in range(2):
                tbl = pool.tile([128, TBL], f32, tag=f"tbl{ct}")
                nc.sync.dma_start(out=tbl[:, 0:T], in_=x[128 * ct:128 * (ct + 1), :])
                for j in range(1, 8):
                    d = 1 << (j - 1)
                    nc.vector.tensor_tensor(
                        out=tbl[:, LVL_OFF[j]:LVL_OFF[j] + LVL_LEN[j]],
                        in0=tbl[:, LVL_OFF[j - 1]:LVL_OFF[j - 1] + LVL_LEN[j]],
                        in1=tbl[:, LVL_OFF[j - 1] + d:LVL_OFF[j - 1] + d + LVL_LEN[j]],
                        op=op.max)
                tbls.append(tbl)

            # --- transpose table to [t, c] in DRAM: PE -> PSUM -> ACT -> DMA ---
            chunks = []  # (t0, m) column chunks of the [c, t]-layout table
            t0 = 0
            while t0 < TBL:
                m = min(128, TBL - t0)
                chunks.append((t0, m))
                t0 += m
            GRP = 4  # transpose chunks per PSUM bank / evac / DMA
            dma_engs = [nc.sync, nc.sync]
            tbl_writes = []
            for ct in range(2):
                for g0 in range(0, len(chunks), GRP):
                    grp = chunks[g0:g0 + GRP]
                    pt = psum.tile([128, 512], f32, tag="pt")
                    ev = pool.tile([128, 512], f32, tag=f"ev{ct}_{g0}")
                    for r, (t0, m) in enumerate(grp):
                        nc.tensor.transpose(
                            pt[0:m, 128 * r:128 * (r + 1)],
                            tbls[ct][:, t0:t0 + m], ident[:, :])
                    # one evac + one DMA per group (fewer, bigger ops)
                    nfull = sum(1 for (_, m) in grp if m == 128)
                    if nfull:
                        nc.scalar.copy(out=ev[:, 0:128 * nfull], in_=pt[:, 0:128 * nfull])
                        t0g = grp[0][0]
                        dst = tbl_dram[t0g:t0g + 128 * nfull, 128 * ct:128 * (ct + 1)]
                        tbl_writes.append(dma_engs[len(tbl_writes) % 2].dma_start(
                            out=dst.rearrange("(a p) c -> p a c", p=128),
                            in_=ev[:, 0:128 * nfull].rearrange("p (a c) -> p a c", a=nfull)))
                    for r, (t0, m) in enumerate(grp):
                        if m != 128:  # ragged tail chunk
                            nc.scalar.copy(out=ev[0:m, 128 * r:128 * (r + 1)],
                                           in_=pt[0:m, 128 * r:128 * (r + 1)])
                            tbl_writes.append(dma_engs[len(tbl_writes) % 2].dma_start(
                                out=tbl_dram[t0:t0 + m, 128 * ct:128 * (ct + 1)],
                                in_=ev[0:m, 128 * r:128 * (r + 1)]))

            # --- fire the prepared gathers once the table is in DRAM.
            # The preps were emitted before the table writes, so Tile's
            # deferred-RAW machinery never saw the producers; wire the
            # trigger's dependency on every table write explicitly. ---
            # Trigger ordering: a tiny DMA read of tbl_dram picks up RAW waits
            # on every table-write's DMA completion; the trigger then declares
            # a WAW "write" of the same tile via signals_writable, so Tile
            # fences it on that DMA's completion (= table fully in DRAM).
            conf = pool.tile([1, 64], f32, tag="conf")
            nc.sync.dma_start(out=conf[:, :], in_=tbl_dram[0:1, 0:64])
            nc.gpsimd.trigger_dma(count=None, signals_writable=[conf[:, :]])
            res = pool.tile([128, 4 * CSH], f32, tag="res")
            nc.vector.tensor_tensor(
                out=res[:, :].rearrange("p (q c) -> p q c", q=4),
                in0=gs[0][:, :, :], in1=gs[1][:, :, :], op=op.max)
            nc.sync.dma_start(out=out[:, 0:2 * CSH], in_=res[:, 0:2 * CSH])
            nc.scalar.dma_start(out=out[:, 2 * CSH:4 * CSH], in_=res[:, 2 * CSH:4 * CSH])
    nc.compile()
    return nc


def _get_nc():
    if "nc" not in _NC_CACHE:
        _NC_CACHE["nc"] = _build()
    return _NC_CACHE["nc"]


def _make_in_maps(input, segments):
    input = np.ascontiguousarray(input, dtype=np.float32)
    segments = np.ascontiguousarray(segments, dtype=np.float32)
    in_maps = []
    for g in range(8):
        b, ch = g // 2, g % 2
        xs = np.ascontiguousarray(input[b, ch * CSH:(ch + 1) * CSH, :])
        seg = np.zeros((KP, 2), np.float32)
        seg[:K] = segments[b]
        # wrapped layout: tile[q, f] = seg[16f + q]; replicated to all 8
        # 16-partition groups; s in cols 0:32, e in cols 32:64
        ss = np.tile(seg[:, 0].reshape(KP // 16, 16).T, (8, 1))
        ee = np.tile(seg[:, 1].reshape(KP // 16, 16).T, (8, 1))
        sw = np.ascontiguousarray(np.concatenate([ss, ee], axis=1), np.float32)
        in_maps.append({"x": xs, "segw": sw})
    return in_maps


def _assemble(results):
    outf = np.empty((B, C, K), np.float32)
    for g in range(8):
        b, ch = g // 2, g % 2
        # res[p, q, c] = answer for query k = q*128 + p, channel c
        arr = results[g]["out"].reshape(128, 4, CSH)
        ans_kc = arr.transpose(1, 0, 2).reshape(KP, CSH)  # [k, c]
        outf[b, ch * CSH:(ch + 1) * CSH, :] = ans_kc[:K, :].T
    return outf


def kernel(input, segments):
    from concourse.bass_utils import run_bass_kernel_spmd

    nc = _get_nc()
    in_maps = _make_in_maps(input, segments)
    res = run_bass_kernel_spmd(nc, in_maps, list(range(8)))
    return _assemble(res.results)
